# revision 7
# baseline (speedup 1.0000x reference)
"""BERT parallel self-attention on 8 Trainium2 NeuronCores (Bass/Tile).

Self-contained: kernel(**inputs) takes the FULL inputs
  hidden_states [2, 4096, 768] f32, attention_mask [2, 1, 1, 4096] f32,
  W_qkv [768, 2304] f32, b_qkv [2304] f32
and returns the FULL context output [2, 4096, 768] f32.

Sharding (Megatron-style tensor-parallel over heads + data-parallel over
batch): core c handles batch c//4, heads 3*(c%4)..3*(c%4)+2. Each core runs
an identical SPMD program on its shard; host gathers the 8 outputs.

Host-side prep: hidden is pre-transposed+cast to hsT [768, S] bf16 so the
device loads it with fast contiguous DMAs; W is packed+cast to bf16.

The kernel is ACT(exp)-bound: 50.3M softmax exponentials per core at
1 elem/lane/cycle @1.2GHz is ~328us + per-instruction overhead. Everything
is organized to keep the ACT engine streaming:

  - The additive mask is folded into V as exp(mask) ([V|1] columns scaled by
    exp(m[t]), which also scales the softmax-denominator ones column), so the
    exp has NO per-t-block bias and one ACTIVATE can span score blocks of
    different t-blocks: 1536-elem instructions (3 PSUM banks) instead of
    1024, amortizing the ~260ns fixed ACT overhead.
  - Attention is a flat stream of 768 score "slots" (one [128t x 512q]
    matmul each, heads row-packed in the 128x128 PE array at K=64), chunked
    3 per exp. QKV production (V, K, Q per 512-token chunk) is interleaved
    into the slot stream as deadline-scheduled fragments riding the score
    PSUM ring, so the first exp fires ~5us in and ACT never waits on a
    serial projection phase.
  - ctx matmuls ([V|1]^T es accumulated over t in PSUM) trail the score
    stream by CTX_LAG chunks so step-boundary PSUM swaps never stall the
    in-order PE queue ahead of ACT.
  - PSUM budget (8 banks): score ring 2x3 banks + one [65,512] ctx
    accumulator pair (bufs=1); postprocess transposes ride the freed ctx
    slots, output normalize+DMA streams per 128-token block.
"""

from contextlib import ExitStack

import ml_dtypes
import numpy as np

import concourse.bass as bass
import concourse.mybir as mybir
import concourse.tile as tile
from concourse import bacc
from concourse.bass import ts
from concourse.bass_utils import run_bass_kernel_spmd
from concourse.masks import make_identity

F32 = mybir.dt.float32
BF16 = mybir.dt.bfloat16
EXP = mybir.ActivationFunctionType.Exp

P = 128
HH = 768          # hidden size
HB = HH // P      # 6 h-blocks
NHEAD = 3         # heads per core
HN = 64
FQKV = 576        # packed feature columns per core
QCHUNK = 512
B, S, H = 2, 4096, 768
N_CORES = 8
CSIZE = 3         # score slots per exp instruction (PSUM banks)
CTX_LAG = 4       # ctx matmul chunks trail score chunks by this many


def _build(nc: bass.Bass, S: int = S):
    TB = S // P               # token blocks
    QC = S // QCHUNK          # q chunks
    assert QC % 2 == 0

    hsT_d = nc.dram_tensor("hsT", [HH, S], BF16, kind="ExternalInput").ap()
    w_d = nc.dram_tensor("w", [HH, FQKV], BF16, kind="ExternalInput").ap()
    b_d = nc.dram_tensor("b", [640, 1], F32, kind="ExternalInput").ap()
    bflat_d = nc.dram_tensor("bflat", [1, 640], F32, kind="ExternalInput").ap()
    mask_d = nc.dram_tensor("mask", [S, 1], F32, kind="ExternalInput").ap()
    out_d = nc.dram_tensor("out", [S, NHEAD * HN], F32, kind="ExternalOutput").ap()

    with tile.TileContext(nc) as tc, ExitStack() as st:
        pool_p = st.enter_context(tc.tile_pool(name="persist", bufs=1))
        # PSUM: "sc" tag = 2 x [128,3,512] f32 (6 banks) shared by score
        # chunks AND the QKV-production staging tiles; "ctA"/"ctB" = one
        # [65,512] f32 bank each, also hosting the postprocess transposes.
        pool_sc = st.enter_context(tc.tile_pool(name="sc", bufs=2, space="PSUM"))
        pool_ct = st.enter_context(tc.tile_pool(name="ct", bufs=1, space="PSUM"))
        pool_es = st.enter_context(tc.tile_pool(name="es", bufs=8))
        pool_cts = st.enter_context(tc.tile_pool(name="cts", bufs=2))
        pool_rz = st.enter_context(tc.tile_pool(name="rz", bufs=4))
        pool_ob = st.enter_context(tc.tile_pool(name="ob", bufs=4))

        hT = pool_p.tile([P, HB, S], BF16, tag="hT")
        QT01 = pool_p.tile([P, S], BF16, tag="QT01")
        KT01 = pool_p.tile([P, S], BF16, tag="KT01")
        QT2 = pool_p.tile([P, S], BF16, tag="QT2")
        KT2 = pool_p.tile([P, S], BF16, tag="KT2")
        VZ = pool_p.tile([P, TB, NHEAD, HN + 1], BF16, tag="VZ")
        wb = pool_p.tile([P, HB, FQKV], BF16, tag="wb")
        btile = pool_p.tile([P, 5], F32, tag="btile")
        bvrow = pool_p.tile([1, NHEAD * HN], F32, tag="bvrow")
        bvb = pool_p.tile([P, NHEAD, HN], F32, tag="bvb")
        masks = pool_p.tile([P, TB], F32, tag="masks")
        em = pool_p.tile([P, TB], F32, tag="em")
        ident = pool_p.tile([P, P], F32, tag="ident")
        ones1 = pool_p.tile([1, P], F32, tag="ones1")

        make_identity(nc, ident[:])
        nc.vector.memset(VZ[:, :, :, HN : HN + 1], 1.0)
        nc.vector.memset(ones1[:], 1.0)

        # ---- input DMAs (none on the ACT queue) ----
        for hb in range(HB):
            nc.gpsimd.dma_start(out=wb[:, hb, :], in_=w_d[ts(hb, P), :])
        for fb in range(5):
            nc.gpsimd.dma_start(out=btile[:, fb : fb + 1], in_=b_d[ts(fb, P), :])
        nc.gpsimd.dma_start(out=bvrow[:], in_=bflat_d[:, 384:576])
        for tb in range(TB):
            nc.gpsimd.dma_start(out=masks[:, tb : tb + 1], in_=mask_d[ts(tb, P), :])
        # hidden, quarter-S x hb slices on two queues, token-major order
        SQ = S // 4
        for tq4 in range(4):
            for hb in range(HB):
                eng = nc.sync if hb % 2 == 0 else nc.scalar
                eng.dma_start(
                    out=hT[:, hb, ts(tq4, SQ)], in_=hsT_d[ts(hb, P), ts(tq4, SQ)]
                )

        # exp(mask): folded into [V|1] so score exps need no bias
        nc.scalar.activation(em[:], masks[:], EXP)

        # V-bias broadcast row -> [128, 192] via K=1 matmul (rides sc ring)
        bvps = pool_sc.tile([P, NHEAD, HN], F32, tag="sc")
        nc.tensor.matmul(bvps[:], ones1[:], bvrow[:], start=True, stop=True)
        nc.vector.tensor_copy(bvb[:], bvps[:])

        # ---- QKV production fragments (ride the "sc" PSUM ring) ----
        def frag_v(tb):
            vv = pool_sc.tile([P, NHEAD, HN], F32, tag="sc")
            for hb in range(HB):
                nc.tensor.matmul(
                    vv[:], hT[:, hb, ts(tb, P)], wb[:, hb, 384:576],
                    start=(hb == 0), stop=(hb == HB - 1),
                )
            nc.vector.tensor_tensor(
                VZ[:, tb, :, 0:HN], vv[:], bvb[:], op=mybir.AluOpType.add
            )
            # scale [V|1] rows by exp(mask[t]) (includes the Z ones column)
            nc.vector.tensor_scalar_mul(
                VZ[:, tb, :, :], VZ[:, tb, :, :], em[:, tb : tb + 1]
            )

        def frag_mixed(fb, tq):
            mm = pool_sc.tile([P, QCHUNK], F32, tag="sc")
            for hb in range(HB):
                nc.tensor.matmul(
                    mm[:], wb[:, hb, ts(fb, P)], hT[:, hb, ts(tq, QCHUNK)],
                    start=(hb == 0), stop=(hb == HB - 1),
                )
            dst = ts(tq, QCHUNK)
            if fb == 0:
                nc.vector.tensor_scalar_add(QT01[:, dst], mm[:], btile[:, 0:1])
            elif fb == 1:
                nc.vector.tensor_scalar_add(KT01[:, dst], mm[:], btile[:, 1:2])
            else:
                nc.vector.tensor_scalar_add(
                    QT2[0:HN, dst], mm[0:HN, :], btile[0:HN, 2:3]
                )
                nc.vector.tensor_scalar_add(
                    KT2[HN:P, dst], mm[HN:P, :], btile[HN:P, 2:3]
                )

        def frag_dup2():
            nc.sync.dma_start(out=QT2[HN:P, :], in_=QT2[0:HN, :])
            nc.sync.dma_start(out=KT2[0:HN, :], in_=KT2[HN:P, :])

        # ---- slot list: 768 score matmuls in step order ----
        steps = []
        for qc in range(QC):  # heads 0,1 (partition-paired)
            steps.append((QT01, KT01, (0, qc), (1, qc), 0, 1))
        for qcp in range(QC // 2):  # head 2 (self-paired across q-chunks)
            steps.append((QT2, KT2, (2, 2 * qcp), (2, 2 * qcp + 1), 2, 2))

        slots = []
        for si, (QT, KT, (hA, qcA), (hB, qcB), hvA, hvB) in enumerate(steps):
            for tb in range(TB):
                for half, (h, qc, hv) in enumerate(
                    ((hA, qcA, hvA), (hB, qcB, hvB))
                ):
                    lo = half * HN
                    slots.append(
                        dict(
                            k=KT[lo : lo + HN, ts(tb, P)],
                            q=QT[lo : lo + HN, ts(qc, QCHUNK)],
                            vz=VZ[:, tb, hv, :],
                            key=(si, half), head=h, qc=qc,
                            first=(tb == 0), last=(tb == TB - 1),
                        )
                    )
        n_chunks = len(slots) // CSIZE
        assert len(slots) % CSIZE == 0

        # ---- production schedule: fragment -> deadline score-chunk ----
        def dl_tb(tb):  # chunk index where t-block tb is first consumed
            return max(0, (2 * tb) // CSIZE - 1)

        frags = []  # (deadline_chunk, closure), emitted in list order
        for tb in range(4):
            frags.append((-1, lambda tb=tb: frag_v(tb)))
        frags.append((-1, lambda: frag_mixed(1, 0)))   # K01 chunk 0
        frags.append((-1, lambda: frag_mixed(0, 0)))   # Q01 chunk 0
        for c in range(1, QC):
            for tbl in range(4):
                tb = 4 * c + tbl
                frags.append((dl_tb(tb), lambda tb=tb: frag_v(tb)))
            frags.append((dl_tb(4 * c), lambda c=c: frag_mixed(1, c)))
        for c in range(1, QC):  # Q01(c) needed at step c (slot 64c)
            frags.append(
                (max(0, (64 * c) // CSIZE - 4), lambda c=c: frag_mixed(0, c))
            )
        for c in range(QC):  # Q2K2(c): spread through steps 1..4, dup after
            frags.append((30 + 12 * c, lambda c=c: frag_mixed(2, c)))
        frags.append((30 + 12 * QC, frag_dup2))
        frags.sort(key=lambda f: f[0])

        # ---- postprocess ----
        ct_tiles = {}        # key -> live psum accumulator
        pending_fin = {}     # tag -> (cts, head, qc, earliest_chunk)
        TAGS = {0: "ctA", 1: "ctB"}

        def pp_copy(key):
            ct = ct_tiles.pop(key)
            cts = pool_cts.tile([HN + 1, QCHUNK], F32, tag="cts")
            nc.vector.tensor_copy(cts[:], ct[:])
            return cts

        def pp_finish(tag):
            cts, head, qc, _ = pending_fin.pop(tag)
            for j in range(QCHUNK // P):
                # rides the sc PSUM ring (freed promptly by exp); riding the
                # ct tags would deadlock: the next step's accumulator is
                # allocated before this runs, so the slot release it would
                # wait on sits behind it in the in-order PE queue.
                tp = pool_sc.tile([P, HN + 1], F32, tag="sc", name="tp")
                nc.tensor.transpose(
                    tp[:], cts[:, ts(j, P)], ident[0 : HN + 1, 0 : HN + 1]
                )
                rz = pool_rz.tile([P, 1], F32, tag="rz")
                nc.vector.reciprocal(rz[:], tp[:, HN : HN + 1])
                ob = pool_ob.tile([P, HN], F32, tag="ob")
                nc.vector.tensor_scalar_mul(ob[:], tp[:, 0:HN], rz[:])
                tb_out = qc * (QCHUNK // P) + j
                nc.gpsimd.dma_start(
                    out=out_d[ts(tb_out, P), ts(head, HN)], in_=ob[:]
                )

        # ---- main stream ----
        es_tiles = {}
        fi = 0

        def emit_ctx(j):
            for pos in range(CSIZE):
                slot = slots[j * CSIZE + pos]
                key = slot["key"]
                if key not in ct_tiles:
                    tag = TAGS[key[1]]
                    if tag in pending_fin:
                        pp_finish(tag)
                    ct_tiles[key] = pool_ct.tile(
                        [HN + 1, QCHUNK], F32, tag=tag, name=f"ct_{key[0]}_{key[1]}"
                    )
                nc.tensor.matmul(
                    ct_tiles[key][:], slot["vz"],
                    es_tiles[j][:, pos, :],
                    start=slot["first"], stop=slot["last"],
                    skip_group_check=True,
                )
            for pos in range(CSIZE):
                slot = slots[j * CSIZE + pos]
                if slot["last"]:
                    tag = TAGS[slot["key"][1]]
                    cts = pp_copy(slot["key"])
                    pending_fin[tag] = (
                        cts, slot["head"], slot["qc"], j + CTX_LAG + 2
                    )
            del es_tiles[j]

        for j in range(n_chunks):
            while fi < len(frags) and frags[fi][0] <= j:
                frags[fi][1]()
                fi += 1
            for tag in list(pending_fin):
                if pending_fin[tag][3] <= j:
                    pp_finish(tag)
            sc = pool_sc.tile([P, CSIZE, QCHUNK], F32, tag="sc")
            for pos in range(CSIZE):
                slot = slots[j * CSIZE + pos]
                nc.tensor.matmul(
                    sc[:, pos, :], slot["k"], slot["q"], start=True, stop=True
                )
            es = pool_es.tile([P, CSIZE, QCHUNK], BF16, tag="es")
            nc.scalar.activation(es[:], sc[:], EXP, scale=0.125)
            es_tiles[j] = es
            if j >= CTX_LAG:
                emit_ctx(j - CTX_LAG)
        for j in range(n_chunks - CTX_LAG, n_chunks):
            emit_ctx(j)
        for tag in list(pending_fin):
            pp_finish(tag)


_NC_CACHE = None


def _get_nc():
    global _NC_CACHE
    if _NC_CACHE is None:
        nc = bacc.Bacc(
            "TRN2", target_bir_lowering=False, debug=False, num_devices=N_CORES
        )
        _build(nc)
        nc.compile()
        _NC_CACHE = nc
    return _NC_CACHE


def _shard_inputs(hidden_states, attention_mask, W_qkv, b_qkv):
    in_maps = []
    for c in range(N_CORES):
        b, hg = c // 4, c % 4
        h0 = 3 * hg
        order = [(0, h0), (0, h0 + 1), (768, h0), (768, h0 + 1),
                 (0, h0 + 2), (768, h0 + 2),
                 (1536, h0), (1536, h0 + 1), (1536, h0 + 2)]
        cols = np.concatenate(
            [np.arange(off + h * HN, off + (h + 1) * HN) for off, h in order]
        )
        w = np.ascontiguousarray(W_qkv[:, cols].astype(ml_dtypes.bfloat16))
        bv = np.zeros(640, dtype=np.float32)
        bv[:FQKV] = b_qkv[cols]
        hsT = np.ascontiguousarray(hidden_states[b].T.astype(ml_dtypes.bfloat16))
        in_maps.append(
            {
                "hsT": hsT,
                "w": w,
                "b": bv[:, None].copy(),
                "bflat": bv[None, :].copy(),
                "mask": np.ascontiguousarray(
                    attention_mask[b, 0, 0, :, None], dtype=np.float32
                ),
            }
        )
    return in_maps


def _unshard(results):
    out = np.empty((B, S, H), dtype=np.float32)
    for c, r in enumerate(results):
        b, hg = c // 4, c % 4
        out[b, :, hg * 192 : (hg + 1) * 192] = r["out"]
    return out


def kernel(hidden_states, attention_mask, W_qkv, b_qkv, _trace=False, _tmpdir=None):
    nc = _get_nc()
    in_maps = _shard_inputs(
        np.asarray(hidden_states), np.asarray(attention_mask),
        np.asarray(W_qkv), np.asarray(b_qkv),
    )
    res = run_bass_kernel_spmd(
        nc, in_maps, core_ids=list(range(N_CORES)), trace=_trace, tmpdir=_tmpdir
    )
    out = _unshard(res.results)
    if _trace:
        kernel.last_exec_time_ns = res.exec_time_ns
        kernel.last_results = res
    return out


# revision 17
# speedup vs baseline: 1.0573x; 1.0573x over previous
"""BERT parallel self-attention on 8 Trainium2 NeuronCores (Bass/Tile).

Self-contained: kernel(**inputs) takes the FULL inputs
  hidden_states [2, 4096, 768] f32, attention_mask [2, 1, 1, 4096] f32,
  W_qkv [768, 2304] f32, b_qkv [2304] f32
and returns the FULL context output [2, 4096, 768] f32.

Sharding (Megatron-style tensor-parallel over heads + data-parallel over
batch): core c handles batch c//4, heads 3*(c%4)..3*(c%4)+2. Each core runs
an identical SPMD program on its shard; host gathers the 8 outputs.

Host-side prep: hidden is pre-transposed+cast to hsT [768, S] bf16 so the
device loads it with fast contiguous DMAs; W is packed+cast to bf16.

The kernel is ACT(exp)-bound: 50.3M softmax exponentials per core at
1 elem/lane/cycle @1.2GHz is ~328us + per-instruction overhead. Everything
is organized to keep the ACT engine streaming:

  - The additive mask is folded into V as exp(mask) ([V|1] columns scaled by
    exp(m[t]), which also scales the softmax-denominator ones column), so the
    exp has NO per-t-block bias and one ACTIVATE can span score blocks of
    different t-blocks: 1536-elem instructions (3 PSUM banks) instead of
    1024, amortizing the ~260ns fixed ACT overhead.
  - Attention is a flat stream of 768 score "slots" (one [128t x 512q]
    matmul each, heads row-packed in the 128x128 PE array at K=64), chunked
    3 per exp. QKV production (V, K, Q per 512-token chunk) is interleaved
    into the slot stream as deadline-scheduled fragments riding the score
    PSUM ring, so the first exp fires ~5us in and ACT never waits on a
    serial projection phase.
  - ctx matmuls ([V|1]^T es accumulated over t in PSUM) trail the score
    stream by CTX_LAG chunks so step-boundary PSUM swaps never stall the
    in-order PE queue ahead of ACT.
  - PSUM budget (8 banks): score ring 2x3 banks + one [65,512] ctx
    accumulator pair (bufs=1); postprocess transposes ride the freed ctx
    slots, output normalize+DMA streams per 128-token block.
"""

from contextlib import ExitStack

import ml_dtypes
import numpy as np

import concourse.bass as bass
import concourse.mybir as mybir
import concourse.tile as tile
from concourse import bacc
from concourse.bass import ts
from concourse.bass_utils import run_bass_kernel_spmd
from concourse.masks import make_identity

F32 = mybir.dt.float32
BF16 = mybir.dt.bfloat16
EXP = mybir.ActivationFunctionType.Exp

P = 128
HH = 768          # hidden size
HB = HH // P      # 6 h-blocks
NHEAD = 3         # heads per core
HN = 64
FQKV = 576        # packed feature columns per core
QCHUNK = 512
B, S, H = 2, 4096, 768
N_CORES = 8
CSIZE = 3         # score slots per exp instruction (PSUM banks)
CTX_LAG = 4       # ctx matmul chunks trail score chunks by this many


def _build(nc: bass.Bass, S: int = S):
    TB = S // P               # token blocks
    QC = S // QCHUNK          # q chunks
    assert QC % 2 == 0

    hsT_d = nc.dram_tensor("hsT", [HH, S], BF16, kind="ExternalInput").ap()
    w_d = nc.dram_tensor("w", [HH, FQKV], BF16, kind="ExternalInput").ap()
    b_d = nc.dram_tensor("b", [640, 1], F32, kind="ExternalInput").ap()
    bflat_d = nc.dram_tensor("bflat", [1, 640], F32, kind="ExternalInput").ap()
    mask_d = nc.dram_tensor("mask", [S, 1], F32, kind="ExternalInput").ap()
    out_d = nc.dram_tensor("out", [S, NHEAD * HN], F32, kind="ExternalOutput").ap()

    with tile.TileContext(nc) as tc, ExitStack() as st:
        pool_p = st.enter_context(tc.tile_pool(name="persist", bufs=1))
        # PSUM budget, 16KB/partition exactly:
        #   "sc"  2 x [128,3,512] f32 = 12KB  score-chunk double buffer
        #   "ct"  1 x [65,512]  f32 =  2KB   the single live ctx accumulator
        #   "mm"  1 x 2KB                    QKV production staging + pp
        #                                    transposes (keeps the sc ring
        #                                    clean so ACT never hiccups)
        pool_sc = st.enter_context(tc.tile_pool(name="sc", bufs=2, space="PSUM"))
        pool_ct = st.enter_context(tc.tile_pool(name="ct", bufs=1, space="PSUM"))
        pool_mm = st.enter_context(tc.tile_pool(name="mm", bufs=1, space="PSUM"))
        pool_es = st.enter_context(tc.tile_pool(name="es", bufs=8))
        pool_cts = st.enter_context(tc.tile_pool(name="cts", bufs=2))
        pool_rz = st.enter_context(tc.tile_pool(name="rz", bufs=4))
        pool_ob = st.enter_context(tc.tile_pool(name="ob", bufs=4))

        hT = pool_p.tile([P, HB, S], BF16, tag="hT")
        QT01 = pool_p.tile([P, S], BF16, tag="QT01")
        KT01 = pool_p.tile([P, S], BF16, tag="KT01")
        QT2 = pool_p.tile([P, S], BF16, tag="QT2")
        KT2 = pool_p.tile([P, S], BF16, tag="KT2")
        VZ = pool_p.tile([P, TB, NHEAD, HN + 1], BF16, tag="VZ")
        wb = pool_p.tile([P, HB, FQKV], BF16, tag="wb")
        btile = pool_p.tile([P, 5], F32, tag="btile")
        bvrow = pool_p.tile([1, NHEAD * HN], F32, tag="bvrow")
        bvb = pool_p.tile([P, NHEAD, HN], F32, tag="bvb")
        masks = pool_p.tile([P, TB], F32, tag="masks")
        em = pool_p.tile([P, TB], F32, tag="em")
        ident = pool_p.tile([P, P], F32, tag="ident")
        ones1 = pool_p.tile([1, P], F32, tag="ones1")

        make_identity(nc, ident[:])
        nc.vector.memset(VZ[:, :, :, HN : HN + 1], 1.0)
        nc.vector.memset(ones1[:], 1.0)

        # ---- input DMAs (none on the ACT queue) ----
        for hb in range(HB):
            nc.gpsimd.dma_start(out=wb[:, hb, :], in_=w_d[ts(hb, P), :])
        for fb in range(5):
            nc.gpsimd.dma_start(out=btile[:, fb : fb + 1], in_=b_d[ts(fb, P), :])
        nc.gpsimd.dma_start(out=bvrow[:], in_=bflat_d[:, 384:576])
        for tb in range(TB):
            nc.gpsimd.dma_start(out=masks[:, tb : tb + 1], in_=mask_d[ts(tb, P), :])
        # hidden, quarter-S x hb slices on two queues, token-major order
        SQ = S // 4
        for tq4 in range(4):
            for hb in range(HB):
                eng = nc.sync if hb % 2 == 0 else nc.scalar
                eng.dma_start(
                    out=hT[:, hb, ts(tq4, SQ)], in_=hsT_d[ts(hb, P), ts(tq4, SQ)]
                )

        # exp(mask): folded into [V|1] so score exps need no bias
        nc.scalar.activation(em[:], masks[:], EXP)

        # V-bias broadcast row -> [128, 192] via K=1 matmul
        bvps = pool_mm.tile([P, NHEAD, HN], F32, tag="mm")
        nc.tensor.matmul(bvps[:], ones1[:], bvrow[:], start=True, stop=True)
        nc.vector.tensor_copy(bvb[:], bvps[:])

        # ---- QKV production fragments (ride the "sc" PSUM ring) ----
        def frag_v(tb):
            vv = pool_mm.tile([P, NHEAD, HN], F32, tag="mm")
            for hb in range(HB):
                nc.tensor.matmul(
                    vv[:], hT[:, hb, ts(tb, P)], wb[:, hb, 384:576],
                    start=(hb == 0), stop=(hb == HB - 1),
                )
            nc.vector.tensor_tensor(
                VZ[:, tb, :, 0:HN], vv[:], bvb[:], op=mybir.AluOpType.add
            )
            # scale [V|1] rows by exp(mask[t]) (includes the Z ones column)
            nc.vector.tensor_scalar_mul(
                VZ[:, tb, :, :], VZ[:, tb, :, :], em[:, tb : tb + 1]
            )

        def frag_mixed(fb, tq):
            mm = pool_mm.tile([P, QCHUNK], F32, tag="mm")
            for hb in range(HB):
                nc.tensor.matmul(
                    mm[:], wb[:, hb, ts(fb, P)], hT[:, hb, ts(tq, QCHUNK)],
                    start=(hb == 0), stop=(hb == HB - 1),
                )
            dst = ts(tq, QCHUNK)
            if fb == 0:
                nc.vector.tensor_scalar_add(QT01[:, dst], mm[:], btile[:, 0:1])
            elif fb == 1:
                nc.vector.tensor_scalar_add(KT01[:, dst], mm[:], btile[:, 1:2])
            else:
                nc.vector.tensor_scalar_add(
                    QT2[0:HN, dst], mm[0:HN, :], btile[0:HN, 2:3]
                )
                nc.vector.tensor_scalar_add(
                    KT2[HN:P, dst], mm[HN:P, :], btile[HN:P, 2:3]
                )

        def frag_dup2():
            # head-2 Q lands at partitions 0-63 from the f-block-2 matmul;
            # its K at 64-127. Scores need both operands on the same rows.
            nc.sync.dma_start(out=QT2[HN:P, :], in_=QT2[0:HN, :])

        # ---- slot list: 768 score matmuls, one (head, qc) step at a time
        # (single live ctx accumulator -> 2KB PSUM for it) ----
        steps = []
        for qc in range(QC):  # heads 0 (PE rows 0-63) and 1 (rows 64-127)
            steps.append((QT01, KT01, 0, qc, 0, 0))
            steps.append((QT01, KT01, 1, qc, 1, HN))
        for qc in range(QC):  # head 2 duplicated to rows 64-127
            steps.append((QT2, KT2, 2, qc, 2, HN))

        slots = []
        for si, (QT, KT, h, qc, hv, lo) in enumerate(steps):
            for tb in range(TB):
                slots.append(
                    dict(
                        k=KT[lo : lo + HN, ts(tb, P)],
                        q=QT[lo : lo + HN, ts(qc, QCHUNK)],
                        vz=VZ[:, tb, hv, :],
                        key=si, head=h, qc=qc,
                        first=(tb == 0), last=(tb == TB - 1),
                    )
                )
        n_chunks = len(slots) // CSIZE
        assert len(slots) % CSIZE == 0

        # ---- production schedule: fragment -> deadline score-chunk ----
        def dl_sc(tb):  # chunk whose scores first consume t-block tb
            return max(0, tb // CSIZE - 1)

        frags = []  # (deadline_chunk, closure), emitted in list order
        for tb in range(4):
            frags.append((-1, lambda tb=tb: frag_v(tb)))
        frags.append((-1, lambda: frag_mixed(1, 0)))   # K01 chunk 0
        frags.append((-1, lambda: frag_mixed(0, 0)))   # Q01 chunk 0
        for c in range(1, QC):
            frags.append((dl_sc(4 * c), lambda c=c: frag_mixed(1, c)))
            for tbl in range(4):
                tb = 4 * c + tbl
                # V is only needed by the (lagged) ctx matmuls
                frags.append(
                    (dl_sc(tb) + CTX_LAG - 1, lambda tb=tb: frag_v(tb))
                )
        for c in range(1, QC):  # Q01(c) needed at step 2c (slot 64c)
            frags.append(
                (max(0, (64 * c) // CSIZE - 4), lambda c=c: frag_mixed(0, c))
            )
        for c in range(QC):  # Q2K2(c): spread well before head-2 steps
            frags.append((30 + 12 * c, lambda c=c: frag_mixed(2, c)))
        frags.append((30 + 12 * QC, frag_dup2))
        frags.sort(key=lambda f: f[0])

        # ---- postprocess ----
        ct_tiles = {}        # key -> live psum accumulator
        pending_fin = []     # (cts, head, qc, earliest_chunk)

        def pp_copy(key):
            ct = ct_tiles.pop(key)
            cts = pool_cts.tile([HN + 1, QCHUNK], F32, tag="cts")
            nc.vector.tensor_copy(cts[:], ct[:])
            return cts

        def pp_finish():
            cts, head, qc, _ = pending_fin.pop(0)
            for j in range(QCHUNK // P):
                # rides the "mm" PSUM tag (freed promptly); riding the ct
                # tag would deadlock: the next step's accumulator is
                # allocated before this runs, so the slot release it would
                # wait on sits behind it in the in-order PE queue.
                tp = pool_mm.tile([P, HN + 1], F32, tag="mm", name="tp")
                nc.tensor.transpose(
                    tp[:], cts[:, ts(j, P)], ident[0 : HN + 1, 0 : HN + 1]
                )
                rz = pool_rz.tile([P, 1], F32, tag="rz")
                nc.vector.reciprocal(rz[:], tp[:, HN : HN + 1])
                ob = pool_ob.tile([P, HN], F32, tag="ob")
                nc.vector.tensor_scalar_mul(ob[:], tp[:, 0:HN], rz[:])
                tb_out = qc * (QCHUNK // P) + j
                nc.gpsimd.dma_start(
                    out=out_d[ts(tb_out, P), ts(head, HN)], in_=ob[:]
                )

        # ---- main stream ----
        es_tiles = {}
        fi = 0

        def emit_ctx(j):
            for pos in range(CSIZE):
                slot = slots[j * CSIZE + pos]
                key = slot["key"]
                if key not in ct_tiles:
                    ct_tiles[key] = pool_ct.tile(
                        [HN + 1, QCHUNK], F32, tag="ct", name=f"ct_{key}"
                    )
                nc.tensor.matmul(
                    ct_tiles[key][:], slot["vz"],
                    es_tiles[j][:, pos, :],
                    start=slot["first"], stop=slot["last"],
                    skip_group_check=True,
                )
            for pos in range(CSIZE):
                slot = slots[j * CSIZE + pos]
                if slot["last"]:
                    cts = pp_copy(slot["key"])
                    pending_fin.append(
                        (cts, slot["head"], slot["qc"], j + CTX_LAG + 2)
                    )
            del es_tiles[j]

        for j in range(n_chunks):
            while fi < len(frags) and frags[fi][0] <= j:
                frags[fi][1]()
                fi += 1
            while pending_fin and pending_fin[0][3] <= j:
                pp_finish()
            sc = pool_sc.tile([P, CSIZE, QCHUNK], F32, tag="sc")
            for pos in range(CSIZE):
                slot = slots[j * CSIZE + pos]
                nc.tensor.matmul(
                    sc[:, pos, :], slot["k"], slot["q"], start=True, stop=True
                )
            es = pool_es.tile([P, CSIZE, QCHUNK], BF16, tag="es")
            nc.scalar.activation(es[:], sc[:], EXP, scale=0.125)
            es_tiles[j] = es
            if j >= CTX_LAG:
                emit_ctx(j - CTX_LAG)
        for j in range(n_chunks - CTX_LAG, n_chunks):
            emit_ctx(j)
        while pending_fin:
            pp_finish()


_NC_CACHE = None


def _get_nc():
    global _NC_CACHE
    if _NC_CACHE is None:
        nc = bacc.Bacc(
            "TRN2", target_bir_lowering=False, debug=False, num_devices=N_CORES
        )
        _build(nc)
        nc.compile()
        _NC_CACHE = nc
    return _NC_CACHE


def _shard_inputs(hidden_states, attention_mask, W_qkv, b_qkv):
    in_maps = []
    for c in range(N_CORES):
        b, hg = c // 4, c % 4
        h0 = 3 * hg
        order = [(0, h0), (0, h0 + 1), (768, h0), (768, h0 + 1),
                 (0, h0 + 2), (768, h0 + 2),
                 (1536, h0), (1536, h0 + 1), (1536, h0 + 2)]
        cols = np.concatenate(
            [np.arange(off + h * HN, off + (h + 1) * HN) for off, h in order]
        )
        w = np.ascontiguousarray(W_qkv[:, cols].astype(ml_dtypes.bfloat16))
        bv = np.zeros(640, dtype=np.float32)
        bv[:FQKV] = b_qkv[cols]
        hsT = np.ascontiguousarray(hidden_states[b].T.astype(ml_dtypes.bfloat16))
        in_maps.append(
            {
                "hsT": hsT,
                "w": w,
                "b": bv[:, None].copy(),
                "bflat": bv[None, :].copy(),
                "mask": np.ascontiguousarray(
                    attention_mask[b, 0, 0, :, None], dtype=np.float32
                ),
            }
        )
    return in_maps


def _unshard(results):
    out = np.empty((B, S, H), dtype=np.float32)
    for c, r in enumerate(results):
        b, hg = c // 4, c % 4
        out[b, :, hg * 192 : (hg + 1) * 192] = r["out"]
    return out


def kernel(hidden_states, attention_mask, W_qkv, b_qkv, _trace=False, _tmpdir=None):
    nc = _get_nc()
    in_maps = _shard_inputs(
        np.asarray(hidden_states), np.asarray(attention_mask),
        np.asarray(W_qkv), np.asarray(b_qkv),
    )
    res = run_bass_kernel_spmd(
        nc, in_maps, core_ids=list(range(N_CORES)), trace=_trace, tmpdir=_tmpdir
    )
    out = _unshard(res.results)
    if _trace:
        kernel.last_exec_time_ns = res.exec_time_ns
        kernel.last_results = res
    return out
